# revision 11
# baseline (speedup 1.0000x reference)
"""Multi-head attention (B=2, S=2048, D=1024, H=16) on 8 TRN2 NeuronCores.

Sharding: tensor-parallel over heads. Core c owns heads {2c, 2c+1} for both
batches: it projects Q/K/V for its heads (column-sharded wq/wk/wv), computes
its 4 [S,S] attention blocks, and a row-sharded partial of the output
projection (wo). Host sums the 8 wo partials and reassembles the full
[B,H,S,S] attention tensor from per-core packed active tiles (mask-zero
regions are filled with zeros on the host).

On-chip structure per (batch, head) block:
  1. QK^T in transposed layout ([k-part, q-free]) -> exp -> PV matmul with a
     ones-column appended to V, so the PV contraction (K=128) also yields the
     softmax row-sums.
  2. Row-sums are PE-transposed to q-partition layout; reciprocal gives the
     normalizer for the x-path, and Ln(recip) gives a per-partition bias so a
     second QK pass in normal layout can emit *normalized* attention
     probabilities in a single fused ACT op: P = exp(s/8 - ln(rowsum)).
  3. Output projection partial: xT_c = woT_c @ UcT, host-reduced.

Matmuls run in float32r (single-pass PE mode, ~1.6e-4 rel err, 4x faster
than fp32's LOW_HIGH 2-pass mode). Set BASS_KERNEL_F32=1 for full-fp32
matmuls.

The mask input is handled generally: each score region is classified on the
host as fully-active / fully-masked (skipped; zeros on host) / mixed
(additive -8e9 mask tile, deduped, applied on DVE before the exp). For the
causal mask this yields the expected lower-triangle work pattern.
"""

import contextlib
import ctypes
import os
import sys
import types

import numpy as np

import concourse.bass as bass
import concourse.mybir as mybir
import concourse.tile as tile
from concourse import bacc
from concourse.bass_utils import run_bass_kernel_spmd
from concourse.masks import make_identity

# Enable walrus LDWEIGHTS elision for consecutive same-stationary matmuls
# (off by default in concourse; verified correct for this kernel).
if not os.environ.get('BASS_KERNEL_NO_LDWOPT'):
    import concourse.bass_utils as _bu
    if not getattr(_bu, '_ldwopt_patched', False):
        _orig_run_command = _bu.run_command

        def _run_command_ldwopt(argv, **kw):
            argv = ['--enable-ldw-opt=true' if a == '--enable-ldw-opt=false'
                    else a for a in argv]
            return _orig_run_command(argv, **kw)

        _bu.run_command = _run_command_ldwopt
        _bu._ldwopt_patched = True

F32 = mybir.dt.float32
F32R = mybir.dt.float32r
MM_DT = F32 if os.environ.get('BASS_KERNEL_F32') else F32R

B, S, D, H = 2, 2048, 1024, 16
DK = D // H               # 64
NCORES = 8
HC = H // NCORES          # heads per core = 2
CD = HC * DK              # per-core head dim = 128
N = B * S                 # 4096
KT_N = S // 128           # 16 k-tiles per block
QC_N = S // 512           # 4 q-chunks per block
QT_N = S // 128           # 16 q-tiles per block
KC_N = S // 512           # 4 k-chunks per block (normal pass)
SCALE = 1.0 / np.sqrt(DK)
MASKADD = np.float32(-8e9)

_prog_cache = {}


def _install_ntff_hook():
    """Register the axon NTFF profiling hook (missing antenv.axon_hooks stub)."""
    if 'antenv.axon_hooks' in sys.modules:
        return
    so_path = '/opt/axon/libaxon_pjrt.so'
    if not os.path.exists(so_path):
        return
    lib = ctypes.CDLL(so_path)
    if not hasattr(lib, 'axon_start_nrt_profile'):
        return
    lib.axon_start_nrt_profile.argtypes = [ctypes.POINTER(ctypes.c_int64),
                                           ctypes.c_size_t]
    lib.axon_start_nrt_profile.restype = ctypes.c_int64
    lib.axon_stop_nrt_profile.argtypes = [ctypes.c_char_p]
    lib.axon_stop_nrt_profile.restype = ctypes.c_int64

    @contextlib.contextmanager
    def _hook(output_dir, device_ids):
        import jax
        jax.devices()
        if device_ids:
            ids = (ctypes.c_int64 * len(device_ids))(*device_ids)
            rc = lib.axon_start_nrt_profile(ids, len(device_ids))
        else:
            rc = lib.axon_start_nrt_profile(None, 0)
        if rc != 0:
            raise RuntimeError(f'axon_start_nrt_profile rc={rc}')
        try:
            yield
        finally:
            n = lib.axon_stop_nrt_profile(str(output_dir).encode())
            print(f'profile: {n} file(s) written to {output_dir}',
                  file=sys.stderr)

    import antenv  # noqa: F401
    mod = types.ModuleType('antenv.axon_hooks')
    mod.get_axon_ntff_profile_hook = lambda: _hook
    sys.modules['antenv.axon_hooks'] = mod


def _classify_mask(act):
    """act: [S, S] bool (q rows, k cols).

    Returns (cls_t, cls_n, mask_tiles, qt_strip_len, pk_offsets, total_pk):
      cls_t[qc][kt]: QK-T regions [128k x 512q] -> None | -1 | mask tile idx
                     (tile stored transposed, [k-part, q-free])
      cls_n[i][j]:   QK-N regions [128q x 512k] -> None | -1 | mask tile idx
      qt_strip_len[i]: ktiles in qtile i's packed output strip
    """
    mask_tiles = []
    tile_index = {}

    def intern_tile(t):
        key = t.tobytes()
        if key not in tile_index:
            tile_index[key] = len(mask_tiles)
            mask_tiles.append(t)
        return tile_index[key]

    cls_t = []
    for qc in range(QC_N):
        row = []
        for kt in range(KT_N):
            sub = act[qc * 512:(qc + 1) * 512, kt * 128:(kt + 1) * 128]
            if not sub.any():
                row.append(None)
            elif sub.all():
                row.append(-1)
            else:
                row.append(intern_tile(
                    np.where(sub.T, np.float32(0), MASKADD).astype(np.float32)))
        cls_t.append(row)

    strip_len = []
    for i in range(QT_N):
        acts = [kt for kt in range(KT_N)
                if act[i * 128:(i + 1) * 128, kt * 128:(kt + 1) * 128].any()]
        strip_len.append((max(acts) + 1) if acts else 0)

    cls_n = []
    for i in range(QT_N):
        row = []
        for j in range(KC_N):
            sub = act[i * 128:(i + 1) * 128, j * 512:(j + 1) * 512]
            if not sub.any():
                row.append(None)
            elif sub.all():
                row.append(-1)
            else:
                row.append(intern_tile(
                    np.where(sub, np.float32(0), MASKADD).astype(np.float32)))
        cls_n.append(row)

    pk_offsets = {}
    off = 0
    for b in range(B):
        for hh in range(HC):
            for i in range(QT_N):
                pk_offsets[(b, hh, i)] = off
                off += 128 * strip_len[i] * 128
    return cls_t, cls_n, mask_tiles, strip_len, pk_offsets, off


def _build_program(cls_t, cls_n, n_masks, strip_len, pk_offsets, total_pk):
    nc = bacc.Bacc(None, target_bir_lowering=False)

    qT_d = nc.dram_tensor('qT', [D, N], MM_DT, kind='ExternalInput')
    kT_d = nc.dram_tensor('kT', [D, N], MM_DT, kind='ExternalInput')
    vT_d = nc.dram_tensor('vT', [D, N], MM_DT, kind='ExternalInput')
    wqT_d = nc.dram_tensor('wqT', [128, 8, CD], MM_DT, kind='ExternalInput')
    wkT_d = nc.dram_tensor('wkT', [128, 8, CD], MM_DT, kind='ExternalInput')
    wvT_d = nc.dram_tensor('wvT', [128, 8, CD], MM_DT, kind='ExternalInput')
    woT_d = nc.dram_tensor('woT', [CD, D], MM_DT, kind='ExternalInput')
    bq_d = nc.dram_tensor('bq', [CD, 1], F32, kind='ExternalInput')
    bk_d = nc.dram_tensor('bk', [CD, 1], F32, kind='ExternalInput')
    bv_d = nc.dram_tensor('bv', [CD, 1], F32, kind='ExternalInput')
    nm = max(n_masks, 1)
    masks_d = nc.dram_tensor('masks', [nm, 128, 512], F32, kind='ExternalInput')

    attn_d = nc.dram_tensor('attn_pk', [max(total_pk, 128)], F32,
                            kind='ExternalOutput')
    xT_d = nc.dram_tensor('xT', [D, N], F32, kind='ExternalOutput')

    with tile.TileContext(nc) as tc:
        with (
            tc.tile_pool(name='const', bufs=1) as cst,
            tc.tile_pool(name='proj_store', bufs=1) as pstore,
        ):
            ident = cst.tile([128, 128], F32)
            make_identity(nc, ident)
            masks_sb = cst.tile([128, nm, 512], F32)
            nc.sync.dma_start(masks_sb[:], masks_d[:].rearrange('n p f -> p n f'))
            w_sb = {}
            for name, d in (('q', wqT_d), ('k', wkT_d), ('v', wvT_d)):
                w_sb[name] = cst.tile([128, 8, CD], MM_DT, tag=f'w{name}',
                                      name=f'w{name}_sb')
                nc.sync.dma_start(w_sb[name][:], d[:])
            b_sb = {}
            for name, d in (('q', bq_d), ('k', bk_d), ('v', bv_d)):
                b_sb[name] = cst.tile([CD, 1], F32, tag=f'b{name}',
                                      name=f'b{name}_sb')
                nc.sync.dma_start(b_sb[name][:], d[:])
            woT_sb = cst.tile([CD, D], MM_DT)
            nc.sync.dma_start(woT_sb[:], woT_d[:])
            ones32 = cst.tile([128, 2 * KT_N], F32, tag='ones32')
            nc.vector.memset(ones32[:], 1.0)

            QT_sb = pstore.tile([CD, N], MM_DT, tag='QT')
            KT_sb = pstore.tile([CD, N], MM_DT, tag='KT')
            VT_sb = pstore.tile([CD, N], F32, tag='VT')
            Vp_sb = pstore.tile([128, 2 * KT_N, 2 * (DK + 1)], MM_DT, tag='Vp')
            UcT_sb = pstore.tile([CD, N], MM_DT, tag='UcT')

            # ---- projections: PT = W_c @ xT, kch-outer over 8 PSUM banks so
            # consecutive matmuls share the stationary w[kch] (LDW elision) ----
            with tc.tile_pool(name='proj_in', bufs=6) as pin, \
                 tc.tile_pool(name='ppsum', bufs=1, space='PSUM') as ppsum:
                for name, src_d, dst in (('q', qT_d, QT_sb), ('k', kT_d, KT_sb),
                                         ('v', vT_d, VT_sb)):
                    banks = [ppsum.tile([128, 512], F32, tag=f'pp{nch}',
                                        name=f'pp{nch}_{name}')
                             for nch in range(8)]
                    for kch in range(8):
                        for nch in range(8):
                            xin = pin.tile([128, 512], MM_DT, tag=f'{name}in')
                            nc.sync.dma_start(
                                xin[:],
                                src_d[kch * 128:(kch + 1) * 128,
                                      nch * 512:(nch + 1) * 512])
                            nc.tensor.matmul(banks[nch][:],
                                             w_sb[name][:, kch, :], xin[:],
                                             start=(kch == 0), stop=(kch == 7))
                    for nch in range(8):
                        nc.scalar.activation(
                            dst[:, nch * 512:(nch + 1) * 512], banks[nch][:CD, :],
                            mybir.ActivationFunctionType.Identity,
                            bias=b_sb[name][:])

            with (
                tc.tile_pool(name='psS', bufs=2, space='PSUM') as psS,
                tc.tile_pool(name='psU', bufs=2, space='PSUM') as psU,
                tc.tile_pool(name='psT', bufs=2, space='PSUM') as psT,
                tc.tile_pool(name='stx_p', bufs=4) as stxp,
                tc.tile_pool(name='strip_p', bufs=2) as strp,
                tc.tile_pool(name='small', bufs=4) as smp,
            ):
                # V' = [V | 1] per head, normal layout, via PE transpose
                for kt in range(2 * KT_N):
                    tp = psT.tile([128, 128], F32, tag='tp')
                    nc.tensor.transpose(tp[:], VT_sb[:, kt * 128:(kt + 1) * 128],
                                        ident[:])
                    nc.vector.tensor_copy(Vp_sb[:, kt, 0:DK], tp[:, 0:DK])
                    nc.vector.tensor_copy(Vp_sb[:, kt, DK + 1:2 * DK + 1],
                                          tp[:, DK:2 * DK])
                nc.vector.tensor_copy(Vp_sb[:, :, DK:DK + 1],
                                      ones32[:, :, None])
                nc.vector.tensor_copy(Vp_sb[:, :, 2 * DK + 1:2 * DK + 2],
                                      ones32[:, :, None])

                # ---- attention, both heads paired ----
                for b in range(B):
                    recip_all = smp.tile([128, HC, QT_N], F32, tag='recip_all')
                    # phase 1: transposed scores -> exp -> PV (+rowsums)
                    for qc in range(QC_N):
                        actives = [(kt, cls_t[qc][kt]) for kt in range(KT_N)
                                   if cls_t[qc][kt] is not None]
                        na = len(actives)
                        up0 = psU.tile([128, 512], F32, tag='upv', name='up0')
                        up1 = psU.tile([128, 512], F32, tag='upv', name='up1')
                        for j, (kt, cls) in enumerate(actives):
                            sp = psS.tile([128, 1024], F32, tag='sps')
                            for hh in range(HC):
                                hs = hh * DK
                                nc.tensor.matmul(
                                    sp[:, hh * 512:hh * 512 + 512],
                                    KT_sb[hs:hs + DK,
                                          b * S + kt * 128:b * S + (kt + 1) * 128],
                                    QT_sb[hs:hs + DK,
                                          b * S + qc * 512:b * S + (qc + 1) * 512],
                                    start=True, stop=True)
                            if cls >= 0:
                                for hh in range(HC):
                                    nc.vector.tensor_add(
                                        sp[:, hh * 512:hh * 512 + 512],
                                        sp[:, hh * 512:hh * 512 + 512],
                                        masks_sb[:, cls, :])
                            stx = stxp.tile([128, 1024], MM_DT, tag='stx')
                            nc.scalar.activation(
                                stx[:], sp[:],
                                mybir.ActivationFunctionType.Exp, scale=SCALE)
                            for hh, up in ((0, up0), (1, up1)):
                                nc.tensor.matmul(
                                    up[0:DK + 1, :],
                                    Vp_sb[:, b * KT_N + kt,
                                          hh * (DK + 1):(hh + 1) * (DK + 1)],
                                    stx[:, hh * 512:hh * 512 + 512],
                                    start=(j == 0), stop=(j == na - 1))
                        for hh, up in ((0, up0), (1, up1)):
                            hs = hh * DK
                            usb = smp.tile([DK + 1, 512], F32, tag='usb')
                            nc.vector.tensor_copy(usb[:], up[0:DK + 1, :])
                            for f in range(4):
                                i = qc * 4 + f
                                tpu = psT.tile([128, 128], F32, tag='tp')
                                nc.tensor.transpose(
                                    tpu[:, 0:DK + 1],
                                    usb[:, f * 128:(f + 1) * 128],
                                    ident[0:DK + 1, 0:DK + 1])
                                nc.vector.reciprocal(
                                    recip_all[:, hh, i:i + 1],
                                    tpu[:, DK:DK + 1])
                                un = smp.tile([128, DK], F32, tag='un')
                                nc.vector.tensor_scalar_mul(
                                    un[:], tpu[:, 0:DK],
                                    recip_all[:, hh, i:i + 1])
                                tpc = psT.tile([128, 128], F32, tag='tp')
                                nc.tensor.transpose(tpc[0:DK, :], un[:], ident[:])
                                qcol = (b * QT_N + i) * 128
                                nc.vector.tensor_copy(
                                    UcT_sb[hs:hs + DK, qcol:qcol + 128],
                                    tpc[0:DK, :])
                    # phase 2+3 per head: one Ln, then fused-normalized strips
                    for hh in range(HC):
                        hs = hh * DK
                        lnr_all = smp.tile([128, QT_N], F32, tag='lnr_all')
                        nc.scalar.activation(lnr_all[:], recip_all[:, hh, :],
                                             mybir.ActivationFunctionType.Ln)
                        for i in range(QT_N):
                            L = strip_len[i]
                            if L == 0:
                                continue
                            strip = strp.tile([128, S], F32, tag='strip')
                            jcs = list(range((L + 3) // 4))
                            for p in range(0, len(jcs), 2):
                                pair = jcs[p:p + 2]
                                pn = psS.tile([128, 1024], F32, tag='sps')
                                done = []
                                for idx, jc in enumerate(pair):
                                    cls = cls_n[i][jc]
                                    lo = jc * 512
                                    if cls is None:
                                        nc.vector.memset(
                                            strip[:, lo:min(lo + 512, L * 128)],
                                            0.0)
                                        continue
                                    nc.tensor.matmul(
                                        pn[:, idx * 512:idx * 512 + 512],
                                        QT_sb[hs:hs + DK,
                                              b * S + i * 128:b * S + (i + 1) * 128],
                                        KT_sb[hs:hs + DK,
                                              b * S + lo:b * S + lo + 512],
                                        start=True, stop=True)
                                    if cls >= 0:
                                        nc.vector.tensor_add(
                                            pn[:, idx * 512:idx * 512 + 512],
                                            pn[:, idx * 512:idx * 512 + 512],
                                            masks_sb[:, cls, :])
                                    done.append(idx)
                                if done == [0, 1]:
                                    nc.scalar.activation(
                                        strip[:, pair[0] * 512:pair[0] * 512 + 1024],
                                        pn[:],
                                        mybir.ActivationFunctionType.Exp,
                                        scale=SCALE, bias=lnr_all[:, i:i + 1])
                                else:
                                    for idx in done:
                                        nc.scalar.activation(
                                            strip[:, pair[idx] * 512:
                                                  pair[idx] * 512 + 512],
                                            pn[:, idx * 512:idx * 512 + 512],
                                            mybir.ActivationFunctionType.Exp,
                                            scale=SCALE, bias=lnr_all[:, i:i + 1])
                            off = pk_offsets[(b, hh, i)]
                            nc.sync.dma_start(
                                attn_d[off:off + 128 * L * 128].rearrange(
                                    '(p f) -> p f', p=128),
                                strip[:, 0:L * 128])

                # ---- output projection partial, nch-paired ----
                with tc.tile_pool(name='xo_p', bufs=3) as xop:
                    for m in range(D // 128):
                        for np2 in range(N // 1024):
                            xp = psS.tile([128, 1024], F32, tag='sps')
                            for half in range(2):
                                lo = np2 * 1024 + half * 512
                                nc.tensor.matmul(
                                    xp[:, half * 512:half * 512 + 512],
                                    woT_sb[:, m * 128:(m + 1) * 128],
                                    UcT_sb[:, lo:lo + 512],
                                    start=True, stop=True)
                            xo = xop.tile([128, 1024], F32, tag='xo')
                            nc.vector.tensor_copy(xo[:], xp[:])
                            nc.sync.dma_start(
                                xT_d[m * 128:(m + 1) * 128,
                                     np2 * 1024:(np2 + 1) * 1024], xo[:])

    nc.finalize()
    return nc


def _numpy_fallback(q, k, v, mask, wq, bq, wk, bk, wv, bv, wo, bo):
    def split_heads(x):
        return x.reshape(B, S, H, DK).transpose(0, 2, 1, 3)
    query = split_heads(q @ wq.T + bq)
    key_ = split_heads(k @ wk.T + bk)
    value = split_heads(v @ wv.T + bv)
    scores = np.einsum('bhqd,bhkd->bhqk', query, key_) / np.sqrt(DK)
    scores = np.where(np.broadcast_to(mask, scores.shape) == 0,
                      np.float32(-1e9), scores)
    scores = scores - scores.max(axis=-1, keepdims=True)
    e = np.exp(scores)
    attn = (e / e.sum(axis=-1, keepdims=True)).astype(np.float32)
    out = np.einsum('bhqk,bhkd->bhqd', attn, value)
    out = out.transpose(0, 2, 1, 3).reshape(B, S, D)
    x = (out @ wo.T + bo).astype(np.float32)
    return x, attn


def kernel(q, k, v, mask, wq, bq, wk, bk, wv, bv, wo, bo):
    q = np.asarray(q, np.float32)
    k = np.asarray(k, np.float32)
    v = np.asarray(v, np.float32)
    mask = np.asarray(mask)
    wq = np.asarray(wq, np.float32); bq = np.asarray(bq, np.float32)
    wk = np.asarray(wk, np.float32); bk = np.asarray(bk, np.float32)
    wv = np.asarray(wv, np.float32); bv = np.asarray(bv, np.float32)
    wo = np.asarray(wo, np.float32); bo = np.asarray(bo, np.float32)

    m4 = np.broadcast_to(mask, (B, H, S, S))
    act0 = m4[0, 0] != 0
    uniform_mask = all(
        np.array_equal(m4[b, h], m4[0, 0]) for b in range(B) for h in range(H))
    if not uniform_mask or (~act0.any(axis=1)).any():
        return _numpy_fallback(q, k, v, mask, wq, bq, wk, bk, wv, bv, wo, bo)

    key = act0.tobytes()
    if key not in _prog_cache:
        cls = _classify_mask(act0)
        _prog_cache[key] = (cls, _build_program(cls[0], cls[1], len(cls[2]),
                                                cls[3], cls[4], cls[5]))
    (cls_t, cls_n, mask_tiles, strip_len, pk_offsets, total_pk), nc = \
        _prog_cache[key]

    qT = np.ascontiguousarray(q.reshape(N, D).T)
    kT = np.ascontiguousarray(k.reshape(N, D).T)
    vT = np.ascontiguousarray(v.reshape(N, D).T)
    nm = max(len(mask_tiles), 1)
    masks_arr = np.zeros((nm, 128, 512), np.float32)
    for i, t in enumerate(mask_tiles):
        masks_arr[i] = t

    in_maps = []
    for c in range(NCORES):
        rs = slice(CD * c, CD * (c + 1))
        in_maps.append({
            'qT': qT, 'kT': kT, 'vT': vT,
            'wqT': np.ascontiguousarray(
                wq[rs, :].T.reshape(8, 128, CD).transpose(1, 0, 2)),
            'wkT': np.ascontiguousarray(
                wk[rs, :].T.reshape(8, 128, CD).transpose(1, 0, 2)),
            'wvT': np.ascontiguousarray(
                wv[rs, :].T.reshape(8, 128, CD).transpose(1, 0, 2)),
            'woT': np.ascontiguousarray(wo[:, rs].T),
            'bq': bq[rs].reshape(CD, 1).copy(),
            'bk': bk[rs].reshape(CD, 1).copy(),
            'bv': bv[rs].reshape(CD, 1).copy(),
            'masks': masks_arr,
        })

    trace = bool(os.environ.get('BASS_KERNEL_TRACE'))
    if trace:
        _install_ntff_hook()
    res = run_bass_kernel_spmd(nc, in_maps, core_ids=list(range(NCORES)),
                               trace=trace)
    kernel.last_results = res
    kernel.last_exec_time_ns = res.exec_time_ns

    # ---- unshard ----
    attn = np.zeros((B, H, S, S), np.float32)
    xT_sum = np.zeros((D, N), np.float32)
    for c in range(NCORES):
        r = res.results[c]
        xT_sum += r['xT']
        pk = r['attn_pk']
        for b in range(B):
            for hh in range(HC):
                h = HC * c + hh
                for i in range(QT_N):
                    L = strip_len[i]
                    if L == 0:
                        continue
                    off = pk_offsets[(b, hh, i)]
                    attn[b, h, i * 128:(i + 1) * 128, 0:L * 128] = \
                        pk[off:off + 128 * L * 128].reshape(128, L * 128)
    x = (xT_sum.T + bo).reshape(B, S, D).astype(np.float32)
    return x, attn


kernel.last_results = None
kernel.last_exec_time_ns = None


# revision 12
# speedup vs baseline: 1.1859x; 1.1859x over previous
"""Multi-head attention (B=2, S=2048, D=1024, H=16) on 8 TRN2 NeuronCores.

Sharding: tensor-parallel over heads. Core c owns heads {2c, 2c+1} for both
batches: it projects Q/K/V for its heads (column-sharded wq/wk/wv), computes
its 4 [S,S] attention blocks, and a row-sharded partial of the output
projection (wo). Host sums the 8 wo partials and reassembles the full
[B,H,S,S] attention tensor from per-core packed active tiles (mask-zero
regions are filled with zeros on the host).

On-chip structure per (batch, head) block:
  1. QK^T in transposed layout ([k-part, q-free]) -> exp -> PV matmul with a
     ones-column appended to V, so the PV contraction (K=128) also yields the
     softmax row-sums.
  2. Row-sums are PE-transposed to q-partition layout; reciprocal gives the
     normalizer for the x-path, and Ln(recip) gives a per-partition bias so a
     second QK pass in normal layout can emit *normalized* attention
     probabilities in a single fused ACT op: P = exp(s/8 - ln(rowsum)).
  3. Output projection partial: xT_c = woT_c @ UcT, host-reduced.

Matmuls run in float32r (single-pass PE mode, ~1.6e-4 rel err, 4x faster
than fp32's LOW_HIGH 2-pass mode). Set BASS_KERNEL_F32=1 for full-fp32
matmuls.

The mask input is handled generally: each score region is classified on the
host as fully-active / fully-masked (skipped; zeros on host) / mixed
(additive -8e9 mask tile, deduped, applied on DVE before the exp). For the
causal mask this yields the expected lower-triangle work pattern.
"""

import contextlib
import ctypes
import os
import sys
import types

import numpy as np

import concourse.bass as bass
import concourse.mybir as mybir
import concourse.tile as tile
from concourse import bacc
from concourse.bass_utils import run_bass_kernel_spmd
from concourse.masks import make_identity

# Enable walrus LDWEIGHTS elision for consecutive same-stationary matmuls
# (off by default in concourse; verified correct for this kernel).
if not os.environ.get('BASS_KERNEL_NO_LDWOPT'):
    import concourse.bass_utils as _bu
    if not getattr(_bu, '_ldwopt_patched', False):
        _orig_run_command = _bu.run_command

        def _run_command_ldwopt(argv, **kw):
            argv = ['--enable-ldw-opt=true' if a == '--enable-ldw-opt=false'
                    else a for a in argv]
            return _orig_run_command(argv, **kw)

        _bu.run_command = _run_command_ldwopt
        _bu._ldwopt_patched = True

F32 = mybir.dt.float32
F32R = mybir.dt.float32r
MM_DT = F32 if os.environ.get('BASS_KERNEL_F32') else F32R

B, S, D, H = 2, 2048, 1024, 16
DK = D // H               # 64
NCORES = 8
HC = H // NCORES          # heads per core = 2
CD = HC * DK              # per-core head dim = 128
N = B * S                 # 4096
KT_N = S // 128           # 16 k-tiles per block
QC_N = S // 512           # 4 q-chunks per block
QT_N = S // 128           # 16 q-tiles per block
KC_N = S // 512           # 4 k-chunks per block (normal pass)
SCALE = 1.0 / np.sqrt(DK)
MASKADD = np.float32(-8e9)

_prog_cache = {}


def _install_ntff_hook():
    """Register the axon NTFF profiling hook (missing antenv.axon_hooks stub)."""
    if 'antenv.axon_hooks' in sys.modules:
        return
    so_path = '/opt/axon/libaxon_pjrt.so'
    if not os.path.exists(so_path):
        return
    lib = ctypes.CDLL(so_path)
    if not hasattr(lib, 'axon_start_nrt_profile'):
        return
    lib.axon_start_nrt_profile.argtypes = [ctypes.POINTER(ctypes.c_int64),
                                           ctypes.c_size_t]
    lib.axon_start_nrt_profile.restype = ctypes.c_int64
    lib.axon_stop_nrt_profile.argtypes = [ctypes.c_char_p]
    lib.axon_stop_nrt_profile.restype = ctypes.c_int64

    @contextlib.contextmanager
    def _hook(output_dir, device_ids):
        import jax
        jax.devices()
        if device_ids:
            ids = (ctypes.c_int64 * len(device_ids))(*device_ids)
            rc = lib.axon_start_nrt_profile(ids, len(device_ids))
        else:
            rc = lib.axon_start_nrt_profile(None, 0)
        if rc != 0:
            raise RuntimeError(f'axon_start_nrt_profile rc={rc}')
        try:
            yield
        finally:
            n = lib.axon_stop_nrt_profile(str(output_dir).encode())
            print(f'profile: {n} file(s) written to {output_dir}',
                  file=sys.stderr)

    import antenv  # noqa: F401
    mod = types.ModuleType('antenv.axon_hooks')
    mod.get_axon_ntff_profile_hook = lambda: _hook
    sys.modules['antenv.axon_hooks'] = mod


def _classify_mask(act):
    """act: [S, S] bool (q rows, k cols).

    Returns (cls_t, cls_n, mask_tiles, qt_strip_len, pk_offsets, total_pk):
      cls_t[qc][kt]: QK-T regions [128k x 512q] -> None | -1 | mask tile idx
                     (tile stored transposed, [k-part, q-free])
      cls_n[i][j]:   QK-N regions [128q x 512k] -> None | -1 | mask tile idx
      qt_strip_len[i]: ktiles in qtile i's packed output strip
    """
    mask_tiles = []
    tile_index = {}

    def intern_tile(t):
        key = t.tobytes()
        if key not in tile_index:
            tile_index[key] = len(mask_tiles)
            mask_tiles.append(t)
        return tile_index[key]

    cls_t = []
    for qc in range(QC_N):
        row = []
        for kt in range(KT_N):
            sub = act[qc * 512:(qc + 1) * 512, kt * 128:(kt + 1) * 128]
            if not sub.any():
                row.append(None)
            elif sub.all():
                row.append(-1)
            else:
                row.append(intern_tile(
                    np.where(sub.T, np.float32(0), MASKADD).astype(np.float32)))
        cls_t.append(row)

    strip_len = []
    for i in range(QT_N):
        acts = [kt for kt in range(KT_N)
                if act[i * 128:(i + 1) * 128, kt * 128:(kt + 1) * 128].any()]
        strip_len.append((max(acts) + 1) if acts else 0)

    cls_n = []
    for i in range(QT_N):
        row = []
        for j in range(KC_N):
            sub = act[i * 128:(i + 1) * 128, j * 512:(j + 1) * 512]
            if not sub.any():
                row.append(None)
            elif sub.all():
                row.append(-1)
            else:
                row.append(intern_tile(
                    np.where(sub, np.float32(0), MASKADD).astype(np.float32)))
        cls_n.append(row)

    pk_offsets = {}
    off = 0
    for b in range(B):
        for hh in range(HC):
            for i in range(QT_N):
                pk_offsets[(b, hh, i)] = off
                off += 128 * strip_len[i] * 128
    return cls_t, cls_n, mask_tiles, strip_len, pk_offsets, off


def _build_program(cls_t, cls_n, n_masks, strip_len, pk_offsets, total_pk):
    nc = bacc.Bacc(None, target_bir_lowering=False)

    qT_d = nc.dram_tensor('qT', [D, N], MM_DT, kind='ExternalInput')
    kT_d = nc.dram_tensor('kT', [D, N], MM_DT, kind='ExternalInput')
    vT_d = nc.dram_tensor('vT', [D, N], MM_DT, kind='ExternalInput')
    wqT_d = nc.dram_tensor('wqT', [128, 8, CD], MM_DT, kind='ExternalInput')
    wkT_d = nc.dram_tensor('wkT', [128, 8, CD], MM_DT, kind='ExternalInput')
    wvT_d = nc.dram_tensor('wvT', [128, 8, CD], MM_DT, kind='ExternalInput')
    woT_d = nc.dram_tensor('woT', [CD, D], MM_DT, kind='ExternalInput')
    bq_d = nc.dram_tensor('bq', [CD, 1], F32, kind='ExternalInput')
    bk_d = nc.dram_tensor('bk', [CD, 1], F32, kind='ExternalInput')
    bv_d = nc.dram_tensor('bv', [CD, 1], F32, kind='ExternalInput')
    nm = max(n_masks, 1)
    masks_d = nc.dram_tensor('masks', [nm, 128, 512], F32, kind='ExternalInput')

    attn_d = nc.dram_tensor('attn_pk', [max(total_pk, 128)], F32,
                            kind='ExternalOutput')
    xT_d = nc.dram_tensor('xT', [D, N], F32, kind='ExternalOutput')

    with tile.TileContext(nc) as tc:
        with (
            tc.tile_pool(name='const', bufs=1) as cst,
            tc.tile_pool(name='proj_store', bufs=1) as pstore,
            tc.tile_pool(name='proj_in', bufs=8) as pin,
            tc.tile_pool(name='stx_p', bufs=4) as stxp,
            tc.tile_pool(name='strip_p', bufs=2) as strp,
            tc.tile_pool(name='small', bufs=4) as smp,
            tc.tile_pool(name='xo_p', bufs=2) as xop,
            tc.tile_pool(name='psS', bufs=2, space='PSUM') as psS,
            tc.tile_pool(name='psU', bufs=2, space='PSUM') as psU,
            tc.tile_pool(name='psT', bufs=2, space='PSUM') as psT,
        ):
            ident = cst.tile([128, 128], F32)
            make_identity(nc, ident)
            masks_sb = cst.tile([128, nm, 512], F32)
            nc.sync.dma_start(masks_sb[:], masks_d[:].rearrange('n p f -> p n f'))
            w_sb = {}
            for name, d in (('q', wqT_d), ('k', wkT_d), ('v', wvT_d)):
                w_sb[name] = cst.tile([128, 8, CD], MM_DT, tag=f'w{name}',
                                      name=f'w{name}_sb')
                nc.sync.dma_start(w_sb[name][:], d[:])
            b_sb = {}
            for name, d in (('q', bq_d), ('k', bk_d), ('v', bv_d)):
                b_sb[name] = cst.tile([CD, 1], F32, tag=f'b{name}',
                                      name=f'b{name}_sb')
                nc.sync.dma_start(b_sb[name][:], d[:])
            woT_sb = cst.tile([CD, D], MM_DT)
            nc.sync.dma_start(woT_sb[:], woT_d[:])
            ones32 = cst.tile([128, 2 * KT_N], F32, tag='ones32')
            nc.vector.memset(ones32[:], 1.0)

            QT_sb = pstore.tile([CD, N], MM_DT, tag='QT')
            KT_sb = pstore.tile([CD, N], MM_DT, tag='KT')
            VT_sb = pstore.tile([CD, N], F32, tag='VT')
            Vp_sb = pstore.tile([128, 2 * KT_N, 2 * (DK + 1)], MM_DT, tag='Vp')
            UcT_sb = pstore.tile([CD, N], MM_DT, tag='UcT')

            ones_done = False
            for b in range(B):
                # ---- projections for this batch half: v, k, q (nch-paired,
                # kch-inner with same-stationary pairs for LDW elision) ----
                for name, src_d, dst in (('v', vT_d, VT_sb), ('k', kT_d, KT_sb),
                                         ('q', qT_d, QT_sb)):
                    for pr in range(2):            # pairs of 512-col chunks
                        c0 = b * S + pr * 1024
                        pp = psS.tile([128, 1024], F32, tag='sps')
                        for kch in range(8):
                            xin = pin.tile([128, 1024], MM_DT, tag='xin')
                            nc.sync.dma_start(
                                xin[:],
                                src_d[kch * 128:(kch + 1) * 128, c0:c0 + 1024])
                            for half in range(2):
                                nc.tensor.matmul(
                                    pp[:, half * 512:half * 512 + 512],
                                    w_sb[name][:, kch, :],
                                    xin[:, half * 512:half * 512 + 512],
                                    start=(kch == 0), stop=(kch == 7))
                        nc.scalar.activation(
                            dst[:, c0:c0 + 1024], pp[:CD, :],
                            mybir.ActivationFunctionType.Identity,
                            bias=b_sb[name][:])

                # V' = [V | 1] for this batch half via PE transpose
                for kt in range(b * KT_N, (b + 1) * KT_N):
                    tp = psT.tile([128, 128], F32, tag='tp')
                    nc.tensor.transpose(tp[:], VT_sb[:, kt * 128:(kt + 1) * 128],
                                        ident[:])
                    nc.vector.tensor_copy(Vp_sb[:, kt, 0:DK], tp[:, 0:DK])
                    nc.vector.tensor_copy(Vp_sb[:, kt, DK + 1:2 * DK + 1],
                                          tp[:, DK:2 * DK])
                if not ones_done:
                    ones_done = True
                    nc.vector.tensor_copy(Vp_sb[:, :, DK:DK + 1],
                                          ones32[:, :, None])
                    nc.vector.tensor_copy(Vp_sb[:, :, 2 * DK + 1:2 * DK + 2],
                                          ones32[:, :, None])

                # ---- attention for this batch half, both heads paired ----
                recip_all = smp.tile([128, HC, QT_N], F32, tag='recip_all')
                for qc in range(QC_N):
                    actives = [(kt, cls_t[qc][kt]) for kt in range(KT_N)
                               if cls_t[qc][kt] is not None]
                    na = len(actives)
                    up0 = psU.tile([128, 512], F32, tag='upv', name='up0')
                    up1 = psU.tile([128, 512], F32, tag='upv', name='up1')
                    for j, (kt, cls) in enumerate(actives):
                        sp = psS.tile([128, 1024], F32, tag='sps')
                        for hh in range(HC):
                            hs = hh * DK
                            nc.tensor.matmul(
                                sp[:, hh * 512:hh * 512 + 512],
                                KT_sb[hs:hs + DK,
                                      b * S + kt * 128:b * S + (kt + 1) * 128],
                                QT_sb[hs:hs + DK,
                                      b * S + qc * 512:b * S + (qc + 1) * 512],
                                start=True, stop=True)
                        if cls >= 0:
                            for hh in range(HC):
                                nc.vector.tensor_add(
                                    sp[:, hh * 512:hh * 512 + 512],
                                    sp[:, hh * 512:hh * 512 + 512],
                                    masks_sb[:, cls, :])
                        stx = stxp.tile([128, 1024], MM_DT, tag='stx')
                        nc.scalar.activation(
                            stx[:], sp[:],
                            mybir.ActivationFunctionType.Exp, scale=SCALE)
                        for hh, up in ((0, up0), (1, up1)):
                            nc.tensor.matmul(
                                up[0:DK + 1, :],
                                Vp_sb[:, b * KT_N + kt,
                                      hh * (DK + 1):(hh + 1) * (DK + 1)],
                                stx[:, hh * 512:hh * 512 + 512],
                                start=(j == 0), stop=(j == na - 1))
                    for hh, up in ((0, up0), (1, up1)):
                        hs = hh * DK
                        usb = smp.tile([DK + 1, 512], F32, tag='usb')
                        nc.vector.tensor_copy(usb[:], up[0:DK + 1, :])
                        for f in range(4):
                            i = qc * 4 + f
                            tpu = psT.tile([128, 128], F32, tag='tp')
                            nc.tensor.transpose(
                                tpu[:, 0:DK + 1],
                                usb[:, f * 128:(f + 1) * 128],
                                ident[0:DK + 1, 0:DK + 1])
                            nc.vector.reciprocal(
                                recip_all[:, hh, i:i + 1],
                                tpu[:, DK:DK + 1])
                            un = smp.tile([128, DK], F32, tag='un')
                            nc.vector.tensor_scalar_mul(
                                un[:], tpu[:, 0:DK],
                                recip_all[:, hh, i:i + 1])
                            tpc = psT.tile([128, 128], F32, tag='tp')
                            nc.tensor.transpose(tpc[0:DK, :], un[:], ident[:])
                            qcol = (b * QT_N + i) * 128
                            nc.vector.tensor_copy(
                                UcT_sb[hs:hs + DK, qcol:qcol + 128],
                                tpc[0:DK, :])
                # phase 2+3 per head: one Ln, then fused-normalized strips
                for hh in range(HC):
                    hs = hh * DK
                    lnr_all = smp.tile([128, QT_N], F32, tag='lnr_all')
                    nc.scalar.activation(lnr_all[:], recip_all[:, hh, :],
                                         mybir.ActivationFunctionType.Ln)
                    for i in range(QT_N):
                        L = strip_len[i]
                        if L == 0:
                            continue
                        strip = strp.tile([128, S], F32, tag='strip')
                        jcs = list(range((L + 3) // 4))
                        for p in range(0, len(jcs), 2):
                            pair = jcs[p:p + 2]
                            pn = psS.tile([128, 1024], F32, tag='sps')
                            done = []
                            for idx, jc in enumerate(pair):
                                cls = cls_n[i][jc]
                                lo = jc * 512
                                if cls is None:
                                    nc.vector.memset(
                                        strip[:, lo:min(lo + 512, L * 128)],
                                        0.0)
                                    continue
                                nc.tensor.matmul(
                                    pn[:, idx * 512:idx * 512 + 512],
                                    QT_sb[hs:hs + DK,
                                          b * S + i * 128:b * S + (i + 1) * 128],
                                    KT_sb[hs:hs + DK,
                                          b * S + lo:b * S + lo + 512],
                                    start=True, stop=True)
                                if cls >= 0:
                                    nc.vector.tensor_add(
                                        pn[:, idx * 512:idx * 512 + 512],
                                        pn[:, idx * 512:idx * 512 + 512],
                                        masks_sb[:, cls, :])
                                done.append(idx)
                            if done == [0, 1]:
                                nc.scalar.activation(
                                    strip[:, pair[0] * 512:pair[0] * 512 + 1024],
                                    pn[:],
                                    mybir.ActivationFunctionType.Exp,
                                    scale=SCALE, bias=lnr_all[:, i:i + 1])
                            else:
                                for idx in done:
                                    nc.scalar.activation(
                                        strip[:, pair[idx] * 512:
                                              pair[idx] * 512 + 512],
                                        pn[:, idx * 512:idx * 512 + 512],
                                        mybir.ActivationFunctionType.Exp,
                                        scale=SCALE, bias=lnr_all[:, i:i + 1])
                        off = pk_offsets[(b, hh, i)]
                        nc.sync.dma_start(
                            attn_d[off:off + 128 * L * 128].rearrange(
                                '(p f) -> p f', p=128),
                            strip[:, 0:L * 128])

            # ---- output projection partial, nch-paired ----
            for m in range(D // 128):
                for np2 in range(N // 1024):
                    xp = psS.tile([128, 1024], F32, tag='sps')
                    for half in range(2):
                        lo = np2 * 1024 + half * 512
                        nc.tensor.matmul(
                            xp[:, half * 512:half * 512 + 512],
                            woT_sb[:, m * 128:(m + 1) * 128],
                            UcT_sb[:, lo:lo + 512],
                            start=True, stop=True)
                    xo = xop.tile([128, 1024], F32, tag='xo')
                    nc.vector.tensor_copy(xo[:], xp[:])
                    nc.sync.dma_start(
                        xT_d[m * 128:(m + 1) * 128,
                             np2 * 1024:(np2 + 1) * 1024], xo[:])

    nc.finalize()
    return nc


def _numpy_fallback(q, k, v, mask, wq, bq, wk, bk, wv, bv, wo, bo):
    def split_heads(x):
        return x.reshape(B, S, H, DK).transpose(0, 2, 1, 3)
    query = split_heads(q @ wq.T + bq)
    key_ = split_heads(k @ wk.T + bk)
    value = split_heads(v @ wv.T + bv)
    scores = np.einsum('bhqd,bhkd->bhqk', query, key_) / np.sqrt(DK)
    scores = np.where(np.broadcast_to(mask, scores.shape) == 0,
                      np.float32(-1e9), scores)
    scores = scores - scores.max(axis=-1, keepdims=True)
    e = np.exp(scores)
    attn = (e / e.sum(axis=-1, keepdims=True)).astype(np.float32)
    out = np.einsum('bhqk,bhkd->bhqd', attn, value)
    out = out.transpose(0, 2, 1, 3).reshape(B, S, D)
    x = (out @ wo.T + bo).astype(np.float32)
    return x, attn


def kernel(q, k, v, mask, wq, bq, wk, bk, wv, bv, wo, bo):
    q = np.asarray(q, np.float32)
    k = np.asarray(k, np.float32)
    v = np.asarray(v, np.float32)
    mask = np.asarray(mask)
    wq = np.asarray(wq, np.float32); bq = np.asarray(bq, np.float32)
    wk = np.asarray(wk, np.float32); bk = np.asarray(bk, np.float32)
    wv = np.asarray(wv, np.float32); bv = np.asarray(bv, np.float32)
    wo = np.asarray(wo, np.float32); bo = np.asarray(bo, np.float32)

    m4 = np.broadcast_to(mask, (B, H, S, S))
    act0 = m4[0, 0] != 0
    uniform_mask = all(
        np.array_equal(m4[b, h], m4[0, 0]) for b in range(B) for h in range(H))
    if not uniform_mask or (~act0.any(axis=1)).any():
        return _numpy_fallback(q, k, v, mask, wq, bq, wk, bk, wv, bv, wo, bo)

    key = act0.tobytes()
    if key not in _prog_cache:
        cls = _classify_mask(act0)
        _prog_cache[key] = (cls, _build_program(cls[0], cls[1], len(cls[2]),
                                                cls[3], cls[4], cls[5]))
    (cls_t, cls_n, mask_tiles, strip_len, pk_offsets, total_pk), nc = \
        _prog_cache[key]

    qT = np.ascontiguousarray(q.reshape(N, D).T)
    kT = np.ascontiguousarray(k.reshape(N, D).T)
    vT = np.ascontiguousarray(v.reshape(N, D).T)
    nm = max(len(mask_tiles), 1)
    masks_arr = np.zeros((nm, 128, 512), np.float32)
    for i, t in enumerate(mask_tiles):
        masks_arr[i] = t

    in_maps = []
    for c in range(NCORES):
        rs = slice(CD * c, CD * (c + 1))
        in_maps.append({
            'qT': qT, 'kT': kT, 'vT': vT,
            'wqT': np.ascontiguousarray(
                wq[rs, :].T.reshape(8, 128, CD).transpose(1, 0, 2)),
            'wkT': np.ascontiguousarray(
                wk[rs, :].T.reshape(8, 128, CD).transpose(1, 0, 2)),
            'wvT': np.ascontiguousarray(
                wv[rs, :].T.reshape(8, 128, CD).transpose(1, 0, 2)),
            'woT': np.ascontiguousarray(wo[:, rs].T),
            'bq': bq[rs].reshape(CD, 1).copy(),
            'bk': bk[rs].reshape(CD, 1).copy(),
            'bv': bv[rs].reshape(CD, 1).copy(),
            'masks': masks_arr,
        })

    trace = bool(os.environ.get('BASS_KERNEL_TRACE'))
    if trace:
        _install_ntff_hook()
    res = run_bass_kernel_spmd(nc, in_maps, core_ids=list(range(NCORES)),
                               trace=trace)
    kernel.last_results = res
    kernel.last_exec_time_ns = res.exec_time_ns

    # ---- unshard ----
    attn = np.zeros((B, H, S, S), np.float32)
    xT_sum = np.zeros((D, N), np.float32)
    for c in range(NCORES):
        r = res.results[c]
        xT_sum += r['xT']
        pk = r['attn_pk']
        for b in range(B):
            for hh in range(HC):
                h = HC * c + hh
                for i in range(QT_N):
                    L = strip_len[i]
                    if L == 0:
                        continue
                    off = pk_offsets[(b, hh, i)]
                    attn[b, h, i * 128:(i + 1) * 128, 0:L * 128] = \
                        pk[off:off + 128 * L * 128].reshape(128, L * 128)
    x = (xT_sum.T + bo).reshape(B, S, D).astype(np.float32)
    return x, attn


kernel.last_results = None
kernel.last_exec_time_ns = None
